# revision 1
# baseline (speedup 1.0000x reference)
"""GCN (3-layer + global-max-pool + MLP head) on 8 Trainium2 NeuronCores.

Strategy:
  - Nodes sharded contiguously: core k owns nodes [12500k, 12500(k+1)) and
    graphs [64k, 64(k+1)) (graph boundaries align with 12500-node shards).
  - Aggregate-first form of each GCN layer: since the graph op commutes with
    the dense right-multiply, h_next = relu(dinv*(segsum(u)) @ W + b) with
    u = dinv * h gathered per edge (A_hat = D^-1/2 (A+I) D^-1/2).
  - Per-edge gather via gpsimd.dma_gather from a replicated (AllGathered) fp16
    node-feature table in DRAM; segment-sum via one-hot matmuls on TensorE
    (edges sorted by target tile; one-hot built on DVE from iota + is_equal).
  - dinv (pure graph-structure normalization) and all index/sort/pad tables
    are prepared on host; all tensor compute (scales, matmuls, relu, pooling,
    MLP head) runs on device.
"""

import os

import numpy as np

import concourse.bass as bass
import concourse.bacc as bacc
import concourse.mybir as mybir
import concourse.tile as tile
from concourse.bass_utils import run_bass_kernel_spmd

# ---------------- problem constants (hardcoded per contract) ----------------
N = 100000
G = 512
F1 = 75
F2 = 150
F3 = 300
NC = 8
P = 128
NLOC = N // NC           # 12500 real nodes per core
T = (NLOC + P - 1) // P  # 98 tiles per core
NSH = T * P              # 12544 padded shard rows
NTAB = NSH * NC          # 100352 table rows
NQ = 4
QR = NTAB // NQ          # 25088 rows per quartile table (< 32768 for int16)
QSUB = NSH // NQ         # 3136: within-shard sub-slice per quartile
GPC = G // NC            # 64 graphs per core
GT = 7                   # tiles per gather group
NGRP = T // GT           # 14 groups
ELEM1 = 128              # fp16 row elems for 75-feat tables (256B)
ELEM3 = 256              # fp16 row elems for 150-feat tables (512B)

FP16 = mybir.dt.float16
FP32 = mybir.dt.float32
I16 = mybir.dt.int16
RELU = mybir.ActivationFunctionType.Relu


def _wrap_idx(flat):
    """int16 index list -> dma_gather wrapped layout [128, n/16] (replicated x8)."""
    n = flat.shape[0]
    assert n % 16 == 0
    w = np.zeros((P, n // 16), np.int16)
    w16 = flat.reshape(n // 16, 16).T  # [16, n/16]
    for r in range(8):
        w[16 * r:16 * (r + 1), :] = w16
    return w


def _host_prep(x, edge_index, batch):
    """Integer/graph-structure preprocessing. Returns per-core input maps and
    the static structure used to build the (SPMD-uniform) program."""
    row = edge_index[0].astype(np.int64)
    col = edge_index[1].astype(np.int64)
    loop = np.arange(N, dtype=np.int64)
    rowf = np.concatenate([row, loop])
    colf = np.concatenate([col, loop])

    deg = np.bincount(colf, minlength=N).astype(np.float64)
    dinv = np.where(deg > 0, deg ** -0.5, 0.0).astype(np.float32)

    # node id -> (quartile table, row within it). Quartile q's table is the
    # concat over cores of each core's shard rows [q*QSUB, (q+1)*QSUB) —
    # buildable by AllGather, and gatherable with a zero-offset source AP
    # (dma_gather ignores nonzero DRAM slice offsets).
    src_core = rowf // NLOC
    src_loc = rowf % NLOC
    src_q = src_loc // QSUB
    src_i = src_core * QSUB + (src_loc % QSUB)
    core = colf // NLOC
    lloc = colf % NLOC
    tloc = lloc // P
    cpos = lloc % P

    order = np.lexsort((src_q, tloc, core))
    srcq_s = src_q[order]
    srci_s, cpos_s = src_i[order], cpos[order]

    key = (core[order] * T + tloc[order]) * NQ + srcq_s
    cnt = np.bincount(key, minlength=NC * T * NQ).reshape(NC, T, NQ)
    starts = np.zeros(NC * T * NQ + 1, np.int64)
    np.cumsum(cnt.reshape(-1), out=starts[1:])

    # uniform (SPMD) block counts per (t, q): max over cores
    B = np.ceil(cnt / P).astype(np.int64).max(axis=0)   # [T, NQ]
    Bt = B.sum(axis=1)
    cb = np.zeros(T + 1, np.int64)
    np.cumsum(Bt, out=cb[1:])

    BLK = np.zeros((NGRP, NQ), np.int64)
    for g in range(NGRP):
        BLK[g] = B[g * GT:(g + 1) * GT].sum(axis=0)
    gq_cols = BLK * 8                       # int16 cols per (g,q)
    gq_colbase = np.zeros((NGRP + 1, NQ), np.int64)
    np.cumsum(gq_cols, axis=0, out=gq_colbase[1:])
    tot_cols = gq_colbase[-1]

    per_core = []
    for k in range(NC):
        idx_q = [np.zeros(int(tot_cols[q]) * 16, np.int16) for q in range(NQ)]
        colv = np.full((P, int(cb[-1])), -1.0, np.float16)
        for g in range(NGRP):
            qoff = [int(gq_colbase[g, q]) * 16 for q in range(NQ)]
            for t in range(g * GT, (g + 1) * GT):
                tb = 0
                for q in range(NQ):
                    nb = int(B[t, q])
                    if nb == 0:
                        continue
                    s0 = starts[(k * T + t) * NQ + q]
                    s1 = starts[(k * T + t) * NQ + q + 1]
                    n_e = int(s1 - s0)
                    nslot = nb * P
                    assert n_e <= nslot
                    idx_q[q][qoff[q]:qoff[q] + n_e] = srci_s[s0:s1].astype(np.int16)
                    cpos_pad = np.full(nslot, -1.0, np.float16)
                    cpos_pad[:n_e] = cpos_s[s0:s1].astype(np.float16)
                    colv[:, cb[t] + tb: cb[t] + tb + nb] = cpos_pad.reshape(nb, P).T
                    qoff[q] += nslot
                    tb += nb
        m = {f"idx{q}": _wrap_idx(idx_q[q]) for q in range(NQ)}
        m["colv"] = colv
        xs = np.zeros((NSH, F1), np.float32)
        xs[:NLOC] = x[k * NLOC:(k + 1) * NLOC]
        dv = np.zeros(NSH, np.float32)
        dv[:NLOC] = dinv[k * NLOC:(k + 1) * NLOC]
        m["x_loc"] = xs
        m["dinv_pm"] = np.ascontiguousarray(dv.reshape(T, P).T)   # [128, T]
        per_core.append(m)

    # graph slices (local, identical across cores by construction)
    gb = np.searchsorted(batch, np.arange(G + 1), side="left")
    gslice = [(int(gb[j]), int(gb[j + 1])) for j in range(GPC)]
    for k in range(NC):
        for j in range(GPC):
            assert gb[k * GPC + j] - k * NLOC == gslice[j][0]
            assert gb[k * GPC + j + 1] - k * NLOC == gslice[j][1]

    struct = {"B": B, "cb": cb, "BLK": BLK, "gq_colbase": gq_colbase,
              "tot_cols": tot_cols, "gslice": gslice}
    return per_core, struct


def _build_program(struct, weights16, weights32):
    B, cb = struct["B"], struct["cb"]
    BLK, gq_colbase, tot_cols = struct["BLK"], struct["gq_colbase"], struct["tot_cols"]
    gslice = struct["gslice"]

    n_dev = 1 if os.environ.get("GCN_SINGLE") else NC
    nc = bacc.Bacc("TRN2", target_bir_lowering=False, debug=False, num_devices=n_dev,
                   num_swdge_queues=4)

    # ---- I/O ----
    x_loc = nc.dram_tensor("x_loc", [NSH, F1], FP32, kind="ExternalInput")
    dinv_pm_d = nc.dram_tensor("dinv_pm", [P, T], FP32, kind="ExternalInput")
    colv_d = nc.dram_tensor("colv", [P, int(cb[-1])], FP16, kind="ExternalInput")
    idx_d = [nc.dram_tensor(f"idx{q}", [P, int(tot_cols[q])], I16, kind="ExternalInput")
             for q in range(NQ)]
    wd = {}
    for n, v in weights16.items():
        wd[n] = nc.dram_tensor(n, list(v.shape), FP16, kind="ExternalInput")
    for n, v in weights32.items():
        wd[n] = nc.dram_tensor(n, list(v.shape), FP32, kind="ExternalInput")
    out_g = nc.dram_tensor("out_g", [GPC, P], FP32, kind="ExternalOutput")

    # per-quartile shard tensors: AllGather for quartile q can start as soon
    # as the tiles covering that quartile are written (early AG overlap)
    u_shard = [[nc.dram_tensor(f"u{l + 1}_shard{q}", [QSUB, e], FP16)
                for q in range(NQ)]
               for l, e in ((0, ELEM1), (1, ELEM1), (2, ELEM3))]
    u_full = [[nc.dram_tensor(f"u{l + 1}_full{q}", [QR, e], FP16, addr_space="Shared")
               for q in range(NQ)]
              for l, e in ((0, ELEM1), (1, ELEM1), (2, ELEM3))]

    def stage_out(tbl, t, stage):
        """DMA stage tile [P, elem] (node tile t) into quartile shard(s)."""
        r0 = t * P
        while r0 < (t + 1) * P:
            q = r0 // QSUB
            r1 = min((t + 1) * P, (q + 1) * QSUB)
            nc.sync.dma_start(
                out=u_shard[tbl][q].ap()[r0 - q * QSUB:r1 - q * QSUB, :],
                in_=stage[r0 - t * P:r1 - t * P, :])
            r0 = r1

    def ag_table(tbl, q):
        if "ag" in SKIP:
            return
        nc.gpsimd.collective_compute(
            "AllGather", mybir.AluOpType.bypass,
            replica_groups=[list(range(n_dev))],
            ins=[u_shard[tbl][q].ap().opt()],
            outs=[u_full[tbl][q].ap()[:QSUB * n_dev, :].opt()])

    # (src table, src elem, F aggregated, F after W, out elem, out table, W, b)
    LAYER = [(0, ELEM1, F1, F1, ELEM1, 1, "W1", "b1"),
             (1, ELEM1, F1, F2, ELEM3, 2, "W2", "b2"),
             (2, ELEM3, F2, F3, None, None, "W3", None)]
    PHASE = int(os.environ.get("GCN_PHASE", "4"))  # 0=prologue,1..3=layers,4=all
    LAYER = LAYER[:max(0, min(3, PHASE))]
    LAYER = LAYER * int(os.environ.get("GCN_REPEAT", "1"))  # timing-only knob
    SKIP = set(os.environ.get("GCN_SKIP", "").split(","))   # timing-only knob

    AG_AFTER = {((q + 1) * QSUB - 1) // P: q for q in range(NQ)}

    with tile.TileContext(nc) as tc:
        with (
            tc.tile_pool(name="const", bufs=1) as const,
            tc.tile_pool(name="work", bufs=2) as work,
            tc.tile_pool(name="msg", bufs=3) as msgp,
            tc.tile_pool(name="psum", bufs=2, space="PSUM") as psum,
            tc.tile_pool(name="psum1", bufs=1, space="PSUM") as psum1,
        ):
            # ---- resident constants ----
            colv_sb = const.tile([P, int(cb[-1])], FP16)
            nc.sync.dma_start(out=colv_sb[:], in_=colv_d.ap())
            dinv_pm = const.tile([P, T], FP32)
            nc.sync.dma_start(out=dinv_pm[:], in_=dinv_pm_d.ap())
            wsb = {}
            for n, v in {**weights16, **weights32}.items():
                dt = FP16 if n in weights16 else FP32
                wsb[n] = const.tile(list(v.shape), dt, tag=f"w_{n}", name=f"w_{n}")
                nc.sync.dma_start(out=wsb[n][:], in_=wd[n].ap())
            gmax = [const.tile([P, GPC], FP32, tag=f"gmax{c}", name=f"gmax{c}")
                    for c in range(3)]
            for c in range(3):
                nc.vector.memset(gmax[c][:], 0.0)

            # ---- prologue: u1 = dinv * x ----
            for t in range(T):
                xt = work.tile([P, F1], FP32, tag="xt")
                nc.sync.dma_start(out=xt[:], in_=x_loc.ap()[t * P:(t + 1) * P, :])
                stage = work.tile([P, ELEM1], FP16, tag="stage")
                nc.vector.memset(stage[:, F1:], 0.0)
                nc.vector.tensor_scalar_mul(stage[:, :F1], xt[:], dinv_pm[:, t:t + 1])
                stage_out(0, t, stage)
                if t in AG_AFTER:
                    ag_table(0, AG_AFTER[t])

            # ---- layers ----
            for (tbl, elem_in, Fin, Fout, elem_out, tbl_out, wname, bname) in LAYER:
                Wt = wsb[wname]
                last_layer = elem_out is None
                # gather units: (group, tile_start, tile_end) — L3 uses
                # half-groups to halve SBUF message buffers (elem is 2x wider)
                units = []
                for g in range(NGRP):
                    if last_layer:
                        units.append((g, g * GT, g * GT + 4))
                        units.append((g, g * GT + 4, (g + 1) * GT))
                    else:
                        units.append((g, g * GT, (g + 1) * GT))
                for (g, t0, t1) in units:
                    msg_q = []
                    for q in range(NQ):
                        off = int(B[g * GT:t0, q].sum())
                        nb = int(B[t0:t1, q].sum())
                        if nb == 0:
                            msg_q.append(None)
                            continue
                        assert nb * P <= 8192, "dma_gather too large"
                        mt = msgp.tile([P, nb, elem_in], FP16, tag=f"msgq{q}")
                        c0 = int(gq_colbase[g, q]) + off * 8
                        if "gather" in SKIP:
                            nc.gpsimd.memset(mt[:], 0)
                            msg_q.append(mt)
                            continue
                        it = work.tile([P, nb * 8], I16, tag=f"idxt{q}", bufs=3,
                                       name=f"idxt{q}")
                        nc.sync.dma_start(out=it[:], in_=idx_d[q].ap()[:, c0:c0 + nb * 8])
                        nc.gpsimd.dma_gather(
                            out_ap=mt[:],
                            in_ap=u_full[tbl][q].ap(),
                            idxs_ap=it[:],
                            num_idxs=nb * P, num_idxs_reg=nb * P,
                            elem_size=elem_in, single_packet=False,
                            queue_num=q)
                        msg_q.append(mt)
                    for t in range(t0, t1):
                        nmm = int(B[t].sum())
                        if nmm == 0:
                            continue
                        # one-hot for all of this tile's blocks in one DVE op
                        oh = work.tile([P, nmm, P], FP16, tag="oh")
                        cvap = colv_sb[:]
                        iot = wsb["iota"][:]
                        iota_bc = bass.AP(iot.tensor, iot.offset,
                                          [iot.ap[0], [0, nmm], iot.ap[1]])
                        colv_bc = bass.AP(cvap.tensor, cvap.offset + int(cb[t]),
                                          [cvap.ap[0], [1, nmm], [0, P]])
                        if "oh" in SKIP:
                            iota_bc1 = bass.AP(iot.tensor, iot.offset,
                                               [iot.ap[0], [0, 1], iot.ap[1]])
                            colv_bc1 = bass.AP(cvap.tensor, cvap.offset + int(cb[t]),
                                               [cvap.ap[0], [1, 1], [0, P]])
                            nc.vector.tensor_tensor(out=oh[:, :1, :], in0=iota_bc1,
                                                    in1=colv_bc1,
                                                    op=mybir.AluOpType.is_equal)
                        else:
                            nc.vector.tensor_tensor(out=oh[:], in0=iota_bc, in1=colv_bc,
                                                    op=mybir.AluOpType.is_equal)
                        agg = psum.tile([P, Fin], FP32, tag="agg", space="PSUM")
                        tb = 0
                        qoffs = B[t0:t].sum(axis=0)
                        for q in range(NQ):
                            for j in range(int(B[t, q])):
                                if "agg" in SKIP:
                                    if tb == 0:
                                        nc.tensor.matmul(
                                            out=agg[:], lhsT=oh[:, 0, :],
                                            rhs=msg_q[q][:, 0, :Fin],
                                            start=True, stop=True)
                                    tb += 1
                                    continue
                                nc.tensor.matmul(
                                    out=agg[:], lhsT=oh[:, tb, :],
                                    rhs=msg_q[q][:, int(qoffs[q]) + j, :Fin],
                                    start=(tb == 0), stop=(tb == nmm - 1))
                                tb += 1
                        # z = dinv * agg -> fp16
                        zs = work.tile([P, Fin], FP16, tag="zs")
                        nc.vector.tensor_scalar_mul(zs[:], agg[:], dinv_pm[:, t:t + 1])
                        # transpose z -> [Fin, 128]
                        f0 = min(Fin, P)
                        ztp = psum.tile([f0, P], FP16, tag="ztp", space="PSUM")
                        nc.tensor.transpose(ztp[:], zs[:, :f0], wsb["ident16"][:])
                        zT = work.tile([f0, P], FP16, tag="zT")
                        nc.vector.tensor_copy(zT[:], ztp[:])
                        if Fin > P:
                            ztp2 = psum1.tile([Fin - P, P], FP16, tag="ztp2", space="PSUM")
                            nc.tensor.transpose(ztp2[:], zs[:, P:Fin], wsb["ident16"][:])
                            zT2 = work.tile([Fin - P, P], FP16, tag="zT2")
                            nc.vector.tensor_copy(zT2[:], ztp2[:])
                        if not last_layer:
                            # node-major h = z @ W  [128, Fout]
                            hps = psum.tile([P, Fout], FP32, tag="hps", space="PSUM")
                            nc.tensor.matmul(out=hps[:], lhsT=zT[:],
                                             rhs=Wt[:Fin, :Fout], start=True, stop=True)
                            hb = work.tile([P, Fout], FP32, tag="hb")
                            nc.vector.tensor_tensor(out=hb[:], in0=hps[:],
                                                    in1=wsb[bname][:],
                                                    op=mybir.AluOpType.add)
                            stage = work.tile([P, elem_out], FP16, tag="stage2")
                            nc.vector.memset(stage[:, Fout:], 0.0)
                            # relu(h*dinv) == dinv*relu(h) since dinv >= 0
                            nc.scalar.activation(stage[:, :Fout], hb[:], RELU,
                                                 scale=dinv_pm[:, t:t + 1])
                            stage_out(tbl_out, t, stage)
                            if t in AG_AFTER:
                                ag_table(tbl_out, AG_AFTER[t])
                        else:
                            # feature-major h4 chunks + incremental max-pool
                            lo_t, hi_t = t * P, min((t + 1) * P, NLOC)
                            for c in range(3):
                                fps = psum.tile([P, P], FP32, tag="hps", space="PSUM")
                                nc.tensor.matmul(
                                    out=fps[:], lhsT=Wt[:, 0, c * P:(c + 1) * P],
                                    rhs=zT[:], start=True, stop=False)
                                nc.tensor.matmul(
                                    out=fps[:], lhsT=Wt[:Fin - P, 1, c * P:(c + 1) * P],
                                    rhs=zT2[:], start=False, stop=True)
                                h4c = work.tile([P, P], FP16, tag="h4c")
                                nc.scalar.activation(h4c[:], fps[:], RELU,
                                                     bias=wsb["b3pm"][:, c:c + 1])
                                for j in range(GPC):
                                    glo, ghi = gslice[j]
                                    a, b_ = max(glo, lo_t), min(ghi, hi_t)
                                    if a >= b_:
                                        continue
                                    red = work.tile([P, 1], FP32, tag="red")
                                    nc.vector.reduce_max(
                                        out=red[:], in_=h4c[:, a - lo_t:b_ - lo_t],
                                        axis=mybir.AxisListType.X)
                                    nc.vector.tensor_tensor(
                                        out=gmax[c][:, j:j + 1],
                                        in0=gmax[c][:, j:j + 1], in1=red[:],
                                        op=mybir.AluOpType.max)

            # ---- MLP head (feature-major, fp32) ----
            q_sb = []
            for m_i in range(8):
                qp = psum.tile([P, GPC], FP32, tag="agg", space="PSUM")
                for c in range(3):
                    nc.tensor.matmul(
                        out=qp[:],
                        lhsT=wsb["Wg1"][:, c, m_i * P:(m_i + 1) * P],
                        rhs=gmax[c][:], start=(c == 0), stop=(c == 2))
                qs = work.tile([P, GPC], FP32, tag=f"qs{m_i}")
                nc.scalar.activation(qs[:], qp[:], RELU,
                                     bias=wsb["bg1pm"][:, m_i:m_i + 1])
                q_sb.append(qs)
            op_ = psum.tile([P, GPC], FP32, tag="agg", space="PSUM")
            for m_i in range(8):
                nc.tensor.matmul(out=op_[:], lhsT=wsb["Wg2"][:, m_i, :],
                                 rhs=q_sb[m_i][:], start=(m_i == 0), stop=(m_i == 7))
            ofm = work.tile([P, GPC], FP32, tag="ofm")
            nc.vector.tensor_scalar_add(ofm[:], op_[:], wsb["bg2pm"][:, :1])
            otp = psum.tile([GPC, P], FP32, tag="ztp", space="PSUM")
            nc.tensor.transpose(otp[:], ofm[:], wsb["ident32"][:])
            osb = work.tile([GPC, P], FP32, tag="osb")
            nc.vector.tensor_copy(osb[:], otp[:])
            nc.sync.dma_start(out=out_g.ap(), in_=osb[:])

    nc.compile()
    return nc


_CACHE = {}


def kernel(**inputs):
    x = np.asarray(inputs["x"], np.float32)
    edge_index = np.asarray(inputs["edge_index"])
    batch = np.asarray(inputs["batch"])
    W1 = np.asarray(inputs["W1"], np.float32); b1 = np.asarray(inputs["b1"], np.float32)
    W2 = np.asarray(inputs["W2"], np.float32); b2 = np.asarray(inputs["b2"], np.float32)
    W3 = np.asarray(inputs["W3"], np.float32); b3 = np.asarray(inputs["b3"], np.float32)
    Wg1 = np.asarray(inputs["Wg1"], np.float32); bg1 = np.asarray(inputs["bg1"], np.float32)
    Wg2 = np.asarray(inputs["Wg2"], np.float32); bg2 = np.asarray(inputs["bg2"], np.float32)

    per_core, struct = _host_prep(x, edge_index, batch)

    # weight/constant layout prep (partition dim <= 128 everywhere)
    W3p = np.zeros((P, 2, 384), np.float16)           # [fi%128, fi//128, fo]
    W3p[:, 0, :F3] = W3[:P].astype(np.float16)
    W3p[:F2 - P, 1, :F3] = W3[P:].astype(np.float16)
    b3pm = np.zeros((P, 3), np.float32)
    b3pm.T.reshape(-1)[:F3] = b3
    Wg1p = np.zeros((P, 3, 1024), np.float32)         # [fi%128, fi//128, fo]
    Wg1p.reshape(P * 3, 1024, order="F")              # noop guard
    for c in range(3):
        rows = Wg1[c * P:min((c + 1) * P, F3)]
        Wg1p[:rows.shape[0], c, :] = rows
    Wg2p = np.ascontiguousarray(Wg2.reshape(8, P, P).transpose(1, 0, 2))  # [fi%128, fi//128, fo]
    bg1pm = np.ascontiguousarray(bg1.reshape(8, P).T)
    bg2pm = bg2.reshape(1, P).T.copy()

    weights16 = {
        "W1": W1.astype(np.float16),
        "W2": W2.astype(np.float16),
        "W3": W3p,
        "iota": np.tile(np.arange(P, dtype=np.float16)[None, :], (P, 1)),
        "ident16": np.eye(P, dtype=np.float16),
    }
    weights32 = {
        "b1": np.tile(b1.reshape(1, F1), (P, 1)),
        "b2": np.tile(b2.reshape(1, F2), (P, 1)), "b3pm": b3pm,
        "Wg1": Wg1p, "bg1pm": bg1pm, "Wg2": Wg2p, "bg2pm": bg2pm,
        "ident32": np.eye(P, dtype=np.float32),
    }

    if "prog" not in _CACHE:
        _CACHE["prog"] = _build_program(struct, weights16, weights32)
    nc = _CACHE["prog"]

    in_maps = []
    for k in range(NC):
        m = dict(per_core[k])
        m.update(weights16)
        m.update(weights32)
        in_maps.append(m)

    res = run_bass_kernel_spmd(nc, in_maps, core_ids=list(range(NC)))
    out = np.concatenate([res.results[k]["out_g"] for k in range(NC)], axis=0)
    return np.ascontiguousarray(out[:, :], np.float32)

